# revision 17
# baseline (speedup 1.0000x reference)
"""Trainium2 Bass kernel for nn_Attention (dense_transformer, ridge regime).

Computation per batch b:
    scores[s]  = <lstm_output[b,s,:], hidden[b,:]>          # [S]
    w          = softmax(scores)                            # [S]
    attn[h]    = sum_s w[s] * lstm_output[b,s,h]            # [H]
    out[b]     = [hidden[b], attn] @ W_combine.T + b_combine

Sharding: data-parallel over batch B=64 across 8 cores (8 batches/core).
W_combine is passed host-transposed (W.T, [2H, H]) and replicated.

v3 (bf16, all-engine balance): all large operands are cast to bf16 on the
host, halving HBM traffic.  Measured per-[128,1024]-tile costs drove the
split: DVE fused mult 0.55us, any reduce ~1.2us, ACT accum 1.41us, GPS mult
2.56us, PE matmul[*,512] 0.38+0.09us.

Per-core dataflow, per batch (16 s-tiles):
  - DMA: L(b) [128,16,1024] bf16, p-major (contiguous per partition),
    double-buffered, issued in 1MiB quarters; W.T after L(1).
  - products L*hidR: DVE fused-8 mult (tiles 0-7), fused-6 (8-13); GPS
    tensor_mul (14-15).
  - row-sums -> scores: DVE reduce_sum (tiles 0-5), ACT Copy+accum (6-15).
  - softmax: DVE rmax -> PE transpose -> DVE rmax2(neg) -> PE bcast ->
    ACT copy -> ACT exp (bias=-max, Z accumulated per partition).
  - einsum2 on PE: M=8 matmuls, lhsT = per-batch zero-padded [128,8] column
    block of wexp, accumulating all 8 batches into one persistent PSUM pair;
    per-batch Z matmul.
  - projection: hidden half spread over mid-stream PE slack, attn half at
    the tail after the end-of-stream attn transposes.
"""

import numpy as np
import ml_dtypes

import concourse.bass as bass
from concourse import bass_isa, library_config, mybir
from concourse.bass_utils import run_bass_kernel_spmd

F32 = mybir.dt.float32
BF16 = mybir.dt.bfloat16
NPBF16 = ml_dtypes.bfloat16

B, S, H = 64, 2048, 1024
NCORES = 8
BPC = B // NCORES          # batches per core
T = S // 128               # s-tiles per batch
NCH = (2 * H) // 128       # 16 chunks of the combined dim
HCH = H // 128             # 8 chunks of one H

NDVE_MUL = 12              # tiles 0..11 multiplied on DVE (rest on GPS)
NACT_RED = 9               # tiles 0..8 reduced on ACT (rest on DVE)

_cached_nc = None
last_results = None


def _build_program():
    nc = bass.Bass()

    lstm_d = nc.declare_dram_parameter("lstm_output", [BPC, S, H], BF16, isOutput=False)
    hid_d = nc.declare_dram_parameter("hidden", [BPC, H], BF16, isOutput=False)
    wt_d = nc.declare_dram_parameter("w_t", [2 * H, H], BF16, isOutput=False)
    b_d = nc.declare_dram_parameter("b_combine", [H], F32, isOutput=False)
    out_d = nc.declare_dram_parameter("out", [BPC, H], F32, isOutput=True)

    # ---- SBUF ----
    L = [nc.alloc_sbuf_tensor(f"L{i}", [128, T, H], BF16) for i in range(2)]
    WT = nc.alloc_sbuf_tensor("WT", [128, NCH, H], BF16)
    hid_t = nc.alloc_sbuf_tensor("hid", [BPC, H], BF16)
    hid = hid_t.ap()
    bias_t = nc.alloc_sbuf_tensor("bias", [BPC, H], F32)
    bias = bias_t.ap()
    out_t = nc.alloc_sbuf_tensor("out_sb", [BPC, H], F32)
    out_sb = out_t.ap()
    hidR = nc.alloc_sbuf_tensor("hidR", [128, BPC, H], BF16)
    prodP = [nc.alloc_sbuf_tensor(f"prodP{i}", [128, NDVE_MUL, H], BF16)
             for i in range(2)]
    prodG = [nc.alloc_sbuf_tensor(f"prodG{i}", [128, T - NDVE_MUL, H], BF16)
             for i in range(2)]
    dmy = nc.alloc_sbuf_tensor("dmy", [128, 1], BF16)
    CT = nc.alloc_sbuf_tensor("CT", [128, NCH, BPC], BF16)
    wexpP = [nc.alloc_sbuf_tensor(f"wexpP{b}", [128, T, BPC], BF16)
             for b in range(BPC)]
    scores = [nc.alloc_sbuf_tensor(f"scores{b}", [128, T], F32) for b in range(BPC)]
    mp = [nc.alloc_sbuf_tensor(f"mp{b}", [128, 1], F32) for b in range(BPC)]
    zp = [nc.alloc_sbuf_tensor(f"zp{b}", [128, 1], F32) for b in range(BPC)]
    negM = [nc.alloc_sbuf_tensor(f"negM{b}", [128, 1], F32) for b in range(BPC)]
    negM1_t = nc.alloc_sbuf_tensor("negM1s", [1, BPC], F32)
    negM1 = [negM1_t.ap()[0:1, b:b + 1] for b in range(BPC)]
    attn8 = nc.alloc_sbuf_tensor("attn8", [BPC, H], BF16)
    rZrow_t = nc.alloc_sbuf_tensor("rZrow", [1, BPC], F32)
    rZrow = rZrow_t.ap()
    rZv_t = nc.alloc_sbuf_tensor("rZv", [BPC, 1], F32)
    rZv = rZv_t.ap()
    ones_col = nc.alloc_sbuf_tensor("ones_col", [1, 128], F32)
    ones128 = nc.alloc_sbuf_tensor("ones128", [128, 1], F32)
    ident = nc.alloc_sbuf_tensor("ident", [128, 128], F32)
    identB = nc.alloc_sbuf_tensor("identB", [128, 128], BF16)

    # ---- PSUM: 8 banks ----
    e2lo_t = nc.alloc_psum_tensor("e2lo", [BPC, 512], F32)
    e2hi_t = nc.alloc_psum_tensor("e2hi", [BPC, 512], F32)
    pjlo_t = nc.alloc_psum_tensor("pjlo", [BPC, 512], F32)
    pjhi_t = nc.alloc_psum_tensor("pjhi", [BPC, 512], F32)
    stage_t = nc.alloc_psum_tensor("stage", [128, 512], F32)
    stage2_t = nc.alloc_psum_tensor("stage2", [128, 512], F32)
    mpT_t = nc.alloc_psum_tensor("mpT", [1, 128], F32)
    zbank_t = nc.alloc_psum_tensor("zbank", [BPC, 64], F32)
    e2lo, e2hi = e2lo_t.ap(), e2hi_t.ap()
    pjlo, pjhi = pjlo_t.ap(), pjhi_t.ap()
    stage, stage2 = stage_t.ap(), stage2_t.ap()
    mpT = mpT_t.ap()
    negM_bc = stage2_t.ap()[:, 0:1]
    Zps = zbank_t.ap()[0:1, 0:BPC]
    rZvT = zbank_t.ap()[0:BPC, 8:9]
    ctT = [stage_t.ap()[:, 4 * c:4 * (c + 1)].bitcast(BF16) for c in range(HCH)]

    # ---------------- two-pass emission ----------------
    ev = {}
    sems = {}
    counts = {}

    class Prog:
        def __init__(self, name):
            self.name = name
            self.emit = False
            self.eng = None
            self.hwm = {}
            self.auto_drain = name in ("dve", "act", "gps")
            self.first_op = True

        def begin(self, eng=None, emit=False):
            self.emit = emit
            self.eng = eng
            self.hwm = {}
            self.first_op = True

        def wait(self, key):
            if len(key) == 2 and isinstance(key[1], int) and key[0] in (
                    "pe", "dve", "act", "gps", "hid", "bias", "hidr",
                    "l0", "l1", "wt", "outd"):
                sname, val = key
            else:
                if self.emit and key not in ev:
                    raise KeyError(f"wait on unknown event {key}")
                sname, val = ev.get(key, (None, 0))
            if val <= 0 or sname is None:
                return
            if self.hwm.get(sname, -1) >= val:
                return
            self.hwm[sname] = val
            if self.emit:
                self.eng.wait_ge(sems[sname], val)

        def op(self, fn, inc=1, sem=None, drain=None):
            sname = sem or self.name
            counts[sname] = counts.get(sname, 0) + inc
            if self.emit:
                do_drain = self.auto_drain if drain is None else drain
                if do_drain and not self.first_op:
                    self.eng.drain()
                inst = fn()
                inst.then_inc(sems[sname], inc)
            self.first_op = False

        def mark(self, *key, sem=None):
            sname = sem or self.name
            ev[(self.name,) + tuple(key)] = (sname, counts.get(sname, 0))

    DMA, PE, DVE, ACT, GPS = Prog("dma"), Prog("pe"), Prog("dve"), Prog("act"), Prog("gps")

    bias_src = b_d[:]
    bias_bcast = bass.AP(
        tensor=bias_src.tensor,
        offset=bias_src.offset,
        ap=[[0, BPC]] + list(bias_src.ap),
    )

    def prog_gps():
        g = GPS.eng if GPS.emit else None
        GPS.op(lambda: g.memset(ones_col.ap(), 1.0))
        GPS.op(lambda: g.memset(ones128.ap(), 1.0))
        GPS.op(lambda: g.memset(ident.ap(), 0.0))
        GPS.op(lambda: g.affine_select(
            out=ident.ap(), in_=ident.ap(),
            compare_op=mybir.AluOpType.not_equal, fill=1.0, base=0,
            pattern=[[-1, 128]], channel_multiplier=1))
        GPS.op(lambda: g.memset(identB.ap(), 0.0), drain=True)
        GPS.op(lambda: g.affine_select(
            out=identB.ap(), in_=identB.ap(),
            compare_op=mybir.AluOpType.not_equal, fill=1.0, base=0,
            pattern=[[-1, 128]], channel_multiplier=1), drain=True)
        for b in range(BPC):
            GPS.op(lambda b=b: g.memset(wexpP[b].ap(), 0.0), drain=False)
        GPS.mark("setup")
        # per-batch: products for tiles NDVE_MUL..15
        for b in range(BPC):
            GPS.wait(("dma", "Lq", b, 3))
            GPS.wait(("dma", "hidr"))
            if b >= 2:
                GPS.wait(("dve", "redD", b - 2))   # prodG slot reuse
            for j in range(T - NDVE_MUL):
                GPS.op(lambda b=b, j=j: g.tensor_mul(
                    prodG[b % 2].ap()[:, j, :],
                    L[b % 2].ap()[:, NDVE_MUL + j, :],
                    hidR.ap()[:, b, :]), drain=False)
                if j == 1:
                    GPS.mark("gmA", b)
            GPS.mark("gmB", b)

    def prog_dma():
        d = DMA.eng if DMA.emit else None
        DMA.op(lambda: d.dma_start(out=hid, in_=hid_d[:]), inc=16, sem="hid")
        DMA.mark("hid", sem="hid")
        DMA.op(lambda: d.dma_start(out=bias, in_=bias_bcast), inc=16, sem="bias")
        DMA.mark("bias", sem="bias")
        hid_src = hid_d[:]
        hidr_bcast = bass.AP(
            tensor=hid_src.tensor,
            offset=hid_src.offset,
            ap=[[0, 128]] + list(hid_src.ap),
        )
        for b in range(BPC):
            if b >= 2:
                DMA.wait(("pe", "e2", b - 2))
            src = lstm_d[b].rearrange("(p t) h -> p t h", t=T)
            sl = f"l{b % 2}"
            for q in range(4):
                DMA.op(lambda src=src, b=b, q=q: d.dma_start(
                    out=L[b % 2].ap()[:, 4 * q:4 * (q + 1), :],
                    in_=src[:, 4 * q:4 * (q + 1), :]),
                    inc=16, sem=sl)
                DMA.mark("Lq", b, q, sem=sl)
                if b == 0 and q == 0:
                    DMA.op(lambda: d.dma_start(out=hidR.ap(), in_=hidr_bcast),
                           inc=16, sem="hidr")
                    DMA.mark("hidr", sem="hidr")
            DMA.mark("L", b, sem=sl)
            if b == 1:
                wt_src = wt_d[:].rearrange("(c p) n -> p c n", p=128)
                DMA.op(lambda: d.dma_start(out=WT.ap(), in_=wt_src),
                       inc=16, sem="wt")
                DMA.mark("wt", sem="wt")
        DMA.wait(("dve", "bias_hi"))
        DMA.op(lambda: d.dma_start(out=out_d[:], in_=out_sb), inc=16, sem="outd")
        DMA.wait(("outd", counts.get("outd", 0)))

    def prog_pe():
        p = PE.eng if PE.emit else None
        PE.wait(("gps", "setup"))
        PE.wait(("dma", "hid"))
        for c in range(HCH):
            PE.op(lambda c=c: p.transpose(
                ctT[c], hid[0:BPC, c * 128:(c + 1) * 128],
                identB.ap()[0:BPC, 0:BPC]))
        PE.mark("hidT")
        for b in range(BPC):
            PE.wait(("dve", "rmax", b))
            if b >= 1:
                PE.wait(("dve", "rmax2", b - 1))
            PE.op(lambda b=b: p.transpose(mpT, mp[b].ap(), ident.ap()))
            PE.mark("transp", b)
            PE.wait(("dve", "rmax2", b))
            if b >= 1:
                PE.wait(("act", "negMcp", b - 1))
            PE.op(lambda b=b: p.matmul(
                negM_bc, lhsT=ones_col.ap(), rhs=negM1[b],
                start=True, stop=True))
            PE.mark("bcast", b)
            PE.wait(("act", "exp", b))
            PE.op(lambda b=b: p.matmul(
                Zps[0:1, b:b + 1], lhsT=zp[b].ap(), rhs=ones128.ap(),
                start=True, stop=True, skip_group_check=True))
            PE.mark("z", b)
            PE.wait(("dma", "L", b))
            for t in range(T):
                PE.op(lambda b=b, t=t: p.matmul(
                    e2lo[0:BPC, :],
                    lhsT=wexpP[b].ap()[:, t, :],
                    rhs=L[b % 2].ap()[:, t, 0:512],
                    start=(b == 0 and t == 0), stop=(b == BPC - 1 and t == T - 1),
                    skip_group_check=True))
                PE.op(lambda b=b, t=t: p.matmul(
                    e2hi[0:BPC, :],
                    lhsT=wexpP[b].ap()[:, t, :],
                    rhs=L[b % 2].ap()[:, t, 512:1024],
                    start=(b == 0 and t == 0), stop=(b == BPC - 1 and t == T - 1),
                    skip_group_check=True))
            PE.mark("e2", b)
            if 2 <= b <= 5:
                PE.wait(("dma", "wt"))
                PE.wait(("dve", "cth"))
                for c in (2 * (b - 2), 2 * (b - 2) + 1):
                    PE.op(lambda c=c: p.matmul(
                        pjlo[0:BPC, :], lhsT=CT.ap()[:, c, :],
                        rhs=WT.ap()[:, c, 0:512],
                        start=(c == 0), stop=False, skip_group_check=True))
                    PE.op(lambda c=c: p.matmul(
                        pjhi[0:BPC, :], lhsT=CT.ap()[:, c, :],
                        rhs=WT.ap()[:, c, 512:1024],
                        start=(c == 0), stop=False, skip_group_check=True))
                PE.mark("pjh", b)
        # ---- tail ----
        PE.wait(("dve", "recip"))
        PE.op(lambda: p.transpose(rZvT, rZrow, ones128.ap()[0:1, 0:1]))
        PE.mark("rZvT")
        PE.wait(("act", "cphi"))
        PE.wait(("dve", "cth"))
        for c in range(HCH):
            PE.op(lambda c=c: p.transpose(
                ctT[c], attn8.ap()[0:BPC, c * 128:(c + 1) * 128],
                identB.ap()[0:BPC, 0:BPC]))
        PE.mark("attnT")
        PE.wait(("dve", "ctA"))
        for c in range(HCH, NCH):
            PE.op(lambda c=c: p.matmul(
                pjlo[0:BPC, :], lhsT=CT.ap()[:, c, :],
                rhs=WT.ap()[:, c, 0:512],
                start=False, stop=(c == NCH - 1), skip_group_check=True))
            PE.op(lambda c=c: p.matmul(
                pjhi[0:BPC, :], lhsT=CT.ap()[:, c, :],
                rhs=WT.ap()[:, c, 512:1024],
                start=False, stop=(c == NCH - 1), skip_group_check=True))
        PE.mark("projdone")

    def prog_dve():
        v = DVE.eng if DVE.emit else None
        DVE.wait(("pe", "hidT"))
        DVE.op(lambda: v.tensor_copy(
            CT.ap()[:, 0:HCH, :], stage_t.ap()[:, 0:4 * HCH].bitcast(BF16)))
        DVE.mark("cth")
        for b in range(BPC):
            DVE.wait(("dma", "hidr"))
            if b >= 2:
                DVE.wait(("act", "red", b - 2))   # prodP slot reuse
            h4 = hidR.ap()[:, b, :].unsqueeze(1).broadcast_to((128, 4, H))
            for m in range(3):
                DVE.wait(("dma", "Lq", b, m))
                DVE.op(lambda b=b, m=m, h4=h4: v.tensor_mul(
                    prodP[b % 2].ap()[:, 4 * m:4 * (m + 1), :],
                    L[b % 2].ap()[:, 4 * m:4 * (m + 1), :], h4),
                    drain=False)
                DVE.mark(f"mA{m + 1}", b)
            for t in range(NACT_RED, T):
                if t == NDVE_MUL:
                    DVE.wait(("gps", "gmA", b))
                if t == NDVE_MUL + 2:
                    DVE.wait(("gps", "gmB", b))
                if t < NDVE_MUL:
                    srcp = prodP[b % 2].ap()[:, t, :]
                else:
                    srcp = prodG[b % 2].ap()[:, t - NDVE_MUL, :]
                DVE.op(lambda b=b, t=t, srcp=srcp: v.reduce_sum(
                    scores[b].ap()[:, t:t + 1], srcp,
                    axis=mybir.AxisListType.X), drain=False)
            DVE.mark("redD", b)
            DVE.wait(("act", "red", b))
            DVE.op(lambda b=b: v.reduce_max(
                mp[b].ap(), scores[b].ap(), axis=mybir.AxisListType.X))
            DVE.mark("rmax", b)
            DVE.wait(("pe", "transp", b))
            DVE.op(lambda b=b: v.reduce_max(
                negM1[b], mpT, axis=mybir.AxisListType.X, negate=True))
            DVE.mark("rmax2", b)
        # ---- tail ----
        DVE.wait(("pe", "z", BPC - 1))
        DVE.op(lambda: v.reciprocal(rZrow, Zps))
        DVE.mark("recip")
        DVE.wait(("pe", "attnT"))
        DVE.op(lambda: v.tensor_copy(
            CT.ap()[:, HCH:NCH, :], stage_t.ap()[:, 0:4 * HCH].bitcast(BF16)))
        DVE.mark("ctA")
        DVE.wait(("pe", "projdone"))
        DVE.wait(("dma", "bias"))
        DVE.op(lambda: v.tensor_add(out_sb[:, 0:512], pjlo[0:BPC, :], bias[:, 0:512]))
        DVE.mark("bias_lo")
        DVE.op(lambda: v.tensor_add(out_sb[:, 512:1024], pjhi[0:BPC, :],
                                    bias[:, 512:1024]), drain=False)
        DVE.mark("bias_hi")

    def prog_act():
        a = ACT.eng if ACT.emit else None
        Copy = mybir.ActivationFunctionType.Copy
        Exp = mybir.ActivationFunctionType.Exp
        for b in range(BPC):
            # reductions for tiles 0..NACT_RED-1 (start as quarters land)
            for t in range(NACT_RED):
                if t % 4 == 0:
                    ACT.wait(("dve", f"mA{t // 4 + 1}", b))
                ACT.op(lambda b=b, t=t: a.activation(
                    out=dmy.ap().broadcast_to((128, H)),
                    in_=prodP[b % 2].ap()[:, t, :], func=Copy,
                    accum_out=scores[b].ap()[:, t:t + 1]),
                    drain=(t == 0))
            ACT.mark("red", b)
            ACT.wait(("pe", "bcast", b))
            ACT.op(lambda b=b: a.activation(
                out=negM[b].ap(), in_=negM_bc, func=Copy))
            ACT.mark("negMcp", b)
            ACT.op(lambda b=b: a.activation(
                out=wexpP[b].ap()[:, :, b], in_=scores[b].ap(), func=Exp,
                bias=negM[b].ap(), scale=1.0, accum_out=zp[b].ap()))
            ACT.mark("exp", b)
        # ---- tail ----
        ACT.wait(("pe", "rZvT"))
        ACT.op(lambda: a.activation(out=rZv, in_=rZvT, func=Copy))
        ACT.mark("rzv")
        ACT.wait(("pe", "e2", BPC - 1))
        ACT.op(lambda: a.activation(
            out=attn8.ap()[0:BPC, 0:512], in_=e2lo[0:BPC, :],
            func=Copy, scale=rZv))
        ACT.mark("cplo")
        ACT.op(lambda: a.activation(
            out=attn8.ap()[0:BPC, 512:1024], in_=e2hi[0:BPC, :],
            func=Copy, scale=rZv), drain=False)
        ACT.mark("cphi")

    progs = [
        (GPS, prog_gps), (DMA, prog_dma), (PE, prog_pe),
        (DVE, prog_dve), (ACT, prog_act),
    ]

    for pr, fn in progs:
        pr.begin(emit=False)
        fn()

    counts.clear()
    sem_names = ["pe", "dve", "act", "gps", "hid", "bias", "hidr",
                 "l0", "l1", "wt", "outd"]
    with nc.Block() as block:
        for sn in sem_names:
            sems[sn] = nc.alloc_semaphore(name=f"{sn}_sem")

        @block.gpsimd
        def _(eng):
            GPS.begin(eng=eng, emit=True)
            prog_gps()

        @block.sync
        def _(eng):
            DMA.begin(eng=eng, emit=True)
            prog_dma()

        @block.tensor
        def _(eng):
            PE.begin(eng=eng, emit=True)
            prog_pe()

        @block.vector
        def _(eng):
            DVE.begin(eng=eng, emit=True)
            prog_dve()

        @block.scalar
        def _(eng):
            ACT.begin(eng=eng, emit=True)
            prog_act()

    return nc


def kernel(lstm_output, hidden, W_combine, b_combine):
    global _cached_nc, last_results
    lstm_output = np.asarray(lstm_output, dtype=np.float32)
    hidden = np.asarray(hidden, dtype=np.float32)
    W_combine = np.asarray(W_combine, dtype=np.float32)
    b_combine = np.asarray(b_combine, dtype=np.float32)

    if _cached_nc is None:
        _cached_nc = _build_program()
    nc = _cached_nc

    wt_host = np.ascontiguousarray(W_combine.T).astype(NPBF16)
    in_maps = []
    for i in range(NCORES):
        sl = slice(i * BPC, (i + 1) * BPC)
        in_maps.append({
            "lstm_output": np.ascontiguousarray(lstm_output[sl]).astype(NPBF16),
            "hidden": np.ascontiguousarray(hidden[sl]).astype(NPBF16),
            "w_t": wt_host,
            "b_combine": b_combine,
        })
    res = run_bass_kernel_spmd(nc, in_maps, core_ids=list(range(NCORES)))
    last_results = res
    return np.concatenate([res.results[i]["out"] for i in range(NCORES)], axis=0)


# revision 20
# speedup vs baseline: 1.2351x; 1.2351x over previous
"""Trainium2 Bass kernel for nn_Attention (dense_transformer, ridge regime).

Computation per batch b:
    scores[s]  = <lstm_output[b,s,:], hidden[b,:]>          # [S]
    w          = softmax(scores)                            # [S]
    attn[h]    = sum_s w[s] * lstm_output[b,s,h]            # [H]
    out[b]     = [hidden[b], attn] @ W_combine.T + b_combine

Sharding: data-parallel over batch B=64 across 8 cores (8 batches/core).
W_combine is passed host-transposed (W.T, [2H, H]) and replicated.

v3 (bf16, all-engine balance): all large operands are cast to bf16 on the
host, halving HBM traffic.  Measured per-[128,1024]-tile costs drove the
split: DVE fused mult 0.55us, any reduce ~1.2us, ACT accum 1.41us, GPS mult
2.56us, PE matmul[*,512] 0.38+0.09us.

Per-core dataflow, per batch (16 s-tiles):
  - DMA: L(b) [128,16,1024] bf16, p-major (contiguous per partition),
    double-buffered, issued in 1MiB quarters; W.T after L(1).
  - products L*hidR: DVE fused-8 mult (tiles 0-7), fused-6 (8-13); GPS
    tensor_mul (14-15).
  - row-sums -> scores: DVE reduce_sum (tiles 0-5), ACT Copy+accum (6-15).
  - softmax: DVE rmax -> PE transpose -> DVE rmax2(neg) -> PE bcast ->
    ACT copy -> ACT exp (bias=-max, Z accumulated per partition).
  - einsum2 on PE: M=8 matmuls, lhsT = per-batch zero-padded [128,8] column
    block of wexp, accumulating all 8 batches into one persistent PSUM pair;
    per-batch Z matmul.
  - projection: hidden half spread over mid-stream PE slack, attn half at
    the tail after the end-of-stream attn transposes.
"""

import numpy as np
import ml_dtypes

import concourse.bass as bass
from concourse import bass_isa, library_config, mybir
from concourse.bass_utils import run_bass_kernel_spmd

F32 = mybir.dt.float32
BF16 = mybir.dt.bfloat16
NPBF16 = ml_dtypes.bfloat16

B, S, H = 64, 2048, 1024
NCORES = 8
BPC = B // NCORES          # batches per core
T = S // 128               # s-tiles per batch
NCH = (2 * H) // 128       # 16 chunks of the combined dim
HCH = H // 128             # 8 chunks of one H

NDVE_MUL = 14              # tiles 0..13 multiplied on DVE (rest on GPS)
NACT_RED = 10              # tiles 0..9 reduced on ACT (rest on DVE)

_cached_nc = None
last_results = None


def _build_program():
    nc = bass.Bass()

    lstm_d = nc.declare_dram_parameter("lstm_output", [BPC, S, H], BF16, isOutput=False)
    hid_d = nc.declare_dram_parameter("hidden", [BPC, H], BF16, isOutput=False)
    wt_d = nc.declare_dram_parameter("w_t", [2 * H, H], BF16, isOutput=False)
    b_d = nc.declare_dram_parameter("b_combine", [H], F32, isOutput=False)
    out_d = nc.declare_dram_parameter("out", [BPC, H], F32, isOutput=True)

    # ---- SBUF ----
    L = [nc.alloc_sbuf_tensor(f"L{i}", [128, T, H], BF16) for i in range(2)]
    WT = nc.alloc_sbuf_tensor("WT", [128, NCH, H], BF16)
    hid_t = nc.alloc_sbuf_tensor("hid", [BPC, H], BF16)
    hid = hid_t.ap()
    bias_t = nc.alloc_sbuf_tensor("bias", [BPC, H], F32)
    bias = bias_t.ap()
    out_t = nc.alloc_sbuf_tensor("out_sb", [BPC, H], F32)
    out_sb = out_t.ap()
    hidR = nc.alloc_sbuf_tensor("hidR", [128, BPC, H], BF16)
    prodP = [nc.alloc_sbuf_tensor(f"prodP{i}", [128, NDVE_MUL, H], BF16)
             for i in range(2)]
    prodG = [nc.alloc_sbuf_tensor(f"prodG{i}", [128, T - NDVE_MUL, H], BF16)
             for i in range(2)]
    dmy = nc.alloc_sbuf_tensor("dmy", [128, 1], BF16)
    CT = nc.alloc_sbuf_tensor("CT", [128, NCH, BPC], BF16)
    wexpP = [nc.alloc_sbuf_tensor(f"wexpP{b}", [128, T, BPC], BF16)
             for b in range(BPC)]
    scores = [nc.alloc_sbuf_tensor(f"scores{b}", [128, T], F32) for b in range(BPC)]
    mp = [nc.alloc_sbuf_tensor(f"mp{b}", [128, 1], F32) for b in range(BPC)]
    zp = [nc.alloc_sbuf_tensor(f"zp{b}", [128, 1], F32) for b in range(BPC)]
    negM = [nc.alloc_sbuf_tensor(f"negM{b}", [128, 1], F32) for b in range(BPC)]
    negM1_t = nc.alloc_sbuf_tensor("negM1s", [1, BPC], F32)
    negM1 = [negM1_t.ap()[0:1, b:b + 1] for b in range(BPC)]
    attn8 = nc.alloc_sbuf_tensor("attn8", [BPC, H], BF16)
    rZrow_t = nc.alloc_sbuf_tensor("rZrow", [1, BPC], F32)
    rZrow = rZrow_t.ap()
    rZv_t = nc.alloc_sbuf_tensor("rZv", [BPC, 1], F32)
    rZv = rZv_t.ap()
    ones_col = nc.alloc_sbuf_tensor("ones_col", [1, 128], F32)
    ones128 = nc.alloc_sbuf_tensor("ones128", [128, 1], F32)
    ident = nc.alloc_sbuf_tensor("ident", [128, 128], F32)
    identB = nc.alloc_sbuf_tensor("identB", [128, 128], BF16)

    # ---- PSUM: 8 banks ----
    e2lo_t = nc.alloc_psum_tensor("e2lo", [BPC, 512], F32)
    e2hi_t = nc.alloc_psum_tensor("e2hi", [BPC, 512], F32)
    pjlo_t = nc.alloc_psum_tensor("pjlo", [BPC, 512], F32)
    pjhi_t = nc.alloc_psum_tensor("pjhi", [BPC, 512], F32)
    stage_t = nc.alloc_psum_tensor("stage", [128, 512], F32)
    stage2_t = nc.alloc_psum_tensor("stage2", [128, 512], F32)
    mpT_t = nc.alloc_psum_tensor("mpT", [1, 128], F32)
    zbank_t = nc.alloc_psum_tensor("zbank", [BPC, 64], F32)
    e2lo, e2hi = e2lo_t.ap(), e2hi_t.ap()
    pjlo, pjhi = pjlo_t.ap(), pjhi_t.ap()
    stage, stage2 = stage_t.ap(), stage2_t.ap()
    mpT = mpT_t.ap()
    negM_bc = stage2_t.ap()[:, 0:1]
    Zps = zbank_t.ap()[0:1, 0:BPC]
    rZvT = zbank_t.ap()[0:BPC, 8:9]
    ctT = [stage_t.ap()[:, 4 * c:4 * (c + 1)].bitcast(BF16) for c in range(HCH)]

    # ---------------- two-pass emission ----------------
    ev = {}
    sems = {}
    counts = {}

    class Prog:
        def __init__(self, name):
            self.name = name
            self.emit = False
            self.eng = None
            self.hwm = {}
            self.auto_drain = name in ("dve", "act", "gps")
            self.first_op = True

        def begin(self, eng=None, emit=False):
            self.emit = emit
            self.eng = eng
            self.hwm = {}
            self.first_op = True

        def wait(self, key):
            if len(key) == 2 and isinstance(key[1], int) and key[0] in (
                    "pe", "dve", "act", "gps", "hid", "bias", "hidr",
                    "l0", "l1", "wt", "outd"):
                sname, val = key
            else:
                if self.emit and key not in ev:
                    raise KeyError(f"wait on unknown event {key}")
                sname, val = ev.get(key, (None, 0))
            if val <= 0 or sname is None:
                return
            if self.hwm.get(sname, -1) >= val:
                return
            self.hwm[sname] = val
            if self.emit:
                self.eng.wait_ge(sems[sname], val)

        def op(self, fn, inc=1, sem=None, drain=None):
            sname = sem or self.name
            counts[sname] = counts.get(sname, 0) + inc
            if self.emit:
                do_drain = self.auto_drain if drain is None else drain
                if do_drain and not self.first_op:
                    self.eng.drain()
                inst = fn()
                inst.then_inc(sems[sname], inc)
            self.first_op = False

        def mark(self, *key, sem=None):
            sname = sem or self.name
            ev[(self.name,) + tuple(key)] = (sname, counts.get(sname, 0))

    DMA, PE, DVE, ACT, GPS = Prog("dma"), Prog("pe"), Prog("dve"), Prog("act"), Prog("gps")

    bias_src = b_d[:]
    bias_bcast = bass.AP(
        tensor=bias_src.tensor,
        offset=bias_src.offset,
        ap=[[0, BPC]] + list(bias_src.ap),
    )

    def prog_gps():
        g = GPS.eng if GPS.emit else None
        GPS.op(lambda: g.memset(ones_col.ap(), 1.0))
        GPS.op(lambda: g.memset(ones128.ap(), 1.0))
        GPS.op(lambda: g.memset(ident.ap(), 0.0))
        GPS.op(lambda: g.affine_select(
            out=ident.ap(), in_=ident.ap(),
            compare_op=mybir.AluOpType.not_equal, fill=1.0, base=0,
            pattern=[[-1, 128]], channel_multiplier=1))
        GPS.op(lambda: g.memset(identB.ap(), 0.0), drain=True)
        GPS.op(lambda: g.affine_select(
            out=identB.ap(), in_=identB.ap(),
            compare_op=mybir.AluOpType.not_equal, fill=1.0, base=0,
            pattern=[[-1, 128]], channel_multiplier=1), drain=True)
        for b in range(BPC):
            GPS.op(lambda b=b: g.memset(wexpP[b].ap(), 0.0), drain=False)
        GPS.mark("setup")
        # per-batch: products for tiles NDVE_MUL..15
        for b in range(BPC):
            GPS.wait(("dma", "Lq", b, 3))
            GPS.wait(("dma", "hidr"))
            if b >= 2:
                GPS.wait(("dve", "redD", b - 2))   # prodG slot reuse
            for j in range(T - NDVE_MUL):
                GPS.op(lambda b=b, j=j: g.tensor_mul(
                    prodG[b % 2].ap()[:, j, :],
                    L[b % 2].ap()[:, NDVE_MUL + j, :],
                    hidR.ap()[:, b, :]), drain=False)
                if j == 0:
                    GPS.mark("gmA", b)
            GPS.mark("gmB", b)

    def prog_dma():
        d = DMA.eng if DMA.emit else None
        DMA.op(lambda: d.dma_start(out=hid, in_=hid_d[:]), inc=16, sem="hid")
        DMA.mark("hid", sem="hid")
        DMA.op(lambda: d.dma_start(out=bias, in_=bias_bcast), inc=16, sem="bias")
        DMA.mark("bias", sem="bias")
        hid_src = hid_d[:]
        hidr_bcast = bass.AP(
            tensor=hid_src.tensor,
            offset=hid_src.offset,
            ap=[[0, 128]] + list(hid_src.ap),
        )
        for b in range(BPC):
            if b >= 2:
                DMA.wait(("pe", "e2", b - 2))
            src = lstm_d[b].rearrange("(p t) h -> p t h", t=T)
            sl = f"l{b % 2}"
            for q in range(4):
                DMA.op(lambda src=src, b=b, q=q: d.dma_start(
                    out=L[b % 2].ap()[:, 4 * q:4 * (q + 1), :],
                    in_=src[:, 4 * q:4 * (q + 1), :]),
                    inc=16, sem=sl)
                DMA.mark("Lq", b, q, sem=sl)
                if b == 0 and q == 0:
                    DMA.op(lambda: d.dma_start(out=hidR.ap(), in_=hidr_bcast),
                           inc=16, sem="hidr")
                    DMA.mark("hidr", sem="hidr")
            DMA.mark("L", b, sem=sl)
            if b == 1:
                wt_src = wt_d[:].rearrange("(c p) n -> p c n", p=128)
                DMA.op(lambda: d.dma_start(out=WT.ap(), in_=wt_src),
                       inc=16, sem="wt")
                DMA.mark("wt", sem="wt")
        DMA.wait(("dve", "bias_hi"))
        DMA.op(lambda: d.dma_start(out=out_d[:], in_=out_sb), inc=16, sem="outd")
        DMA.wait(("outd", counts.get("outd", 0)))

    def prog_pe():
        p = PE.eng if PE.emit else None
        PE.wait(("gps", "setup"))
        PE.wait(("dma", "hid"))
        for c in range(HCH):
            PE.op(lambda c=c: p.transpose(
                ctT[c], hid[0:BPC, c * 128:(c + 1) * 128],
                identB.ap()[0:BPC, 0:BPC]))
        PE.mark("hidT")
        for b in range(BPC):
            PE.wait(("dve", "rmax", b))
            if b >= 1:
                PE.wait(("dve", "rmax2", b - 1))
            PE.op(lambda b=b: p.transpose(mpT, mp[b].ap(), ident.ap()))
            PE.mark("transp", b)
            PE.wait(("dve", "rmax2", b))
            if b >= 1:
                PE.wait(("act", "negMcp", b - 1))
            PE.op(lambda b=b: p.matmul(
                negM_bc, lhsT=ones_col.ap(), rhs=negM1[b],
                start=True, stop=True))
            PE.mark("bcast", b)
            PE.wait(("act", "exp", b))
            PE.op(lambda b=b: p.matmul(
                Zps[0:1, b:b + 1], lhsT=zp[b].ap(), rhs=ones128.ap(),
                start=True, stop=True, skip_group_check=True))
            PE.mark("z", b)
            PE.wait(("dma", "L", b))
            for t in range(T):
                PE.op(lambda b=b, t=t: p.matmul(
                    e2lo[0:BPC, :],
                    lhsT=wexpP[b].ap()[:, t, :],
                    rhs=L[b % 2].ap()[:, t, 0:512],
                    start=(b == 0 and t == 0), stop=(b == BPC - 1 and t == T - 1),
                    skip_group_check=True))
                PE.op(lambda b=b, t=t: p.matmul(
                    e2hi[0:BPC, :],
                    lhsT=wexpP[b].ap()[:, t, :],
                    rhs=L[b % 2].ap()[:, t, 512:1024],
                    start=(b == 0 and t == 0), stop=(b == BPC - 1 and t == T - 1),
                    skip_group_check=True))
            PE.mark("e2", b)
            if 2 <= b <= 5:
                PE.wait(("dma", "wt"))
                PE.wait(("dve", "cth"))
                for c in (2 * (b - 2), 2 * (b - 2) + 1):
                    PE.op(lambda c=c: p.matmul(
                        pjlo[0:BPC, :], lhsT=CT.ap()[:, c, :],
                        rhs=WT.ap()[:, c, 0:512],
                        start=(c == 0), stop=False, skip_group_check=True))
                    PE.op(lambda c=c: p.matmul(
                        pjhi[0:BPC, :], lhsT=CT.ap()[:, c, :],
                        rhs=WT.ap()[:, c, 512:1024],
                        start=(c == 0), stop=False, skip_group_check=True))
                PE.mark("pjh", b)
        # ---- tail ----
        PE.wait(("dve", "recip"))
        PE.op(lambda: p.transpose(rZvT, rZrow, ones128.ap()[0:1, 0:1]))
        PE.mark("rZvT")
        PE.wait(("act", "cphi"))
        PE.wait(("dve", "cth"))
        for c in range(HCH):
            PE.op(lambda c=c: p.transpose(
                ctT[c], attn8.ap()[0:BPC, c * 128:(c + 1) * 128],
                identB.ap()[0:BPC, 0:BPC]))
        PE.mark("attnT")
        PE.wait(("dve", "ctA"))
        for c in range(HCH, NCH):
            PE.op(lambda c=c: p.matmul(
                pjlo[0:BPC, :], lhsT=CT.ap()[:, c, :],
                rhs=WT.ap()[:, c, 0:512],
                start=False, stop=(c == NCH - 1), skip_group_check=True))
            PE.op(lambda c=c: p.matmul(
                pjhi[0:BPC, :], lhsT=CT.ap()[:, c, :],
                rhs=WT.ap()[:, c, 512:1024],
                start=False, stop=(c == NCH - 1), skip_group_check=True))
        PE.mark("projdone")

    def prog_dve():
        v = DVE.eng if DVE.emit else None
        DVE.wait(("pe", "hidT"))
        DVE.op(lambda: v.tensor_copy(
            CT.ap()[:, 0:HCH, :], stage_t.ap()[:, 0:4 * HCH].bitcast(BF16)))
        DVE.mark("cth")
        for b in range(BPC):
            DVE.wait(("dma", "hidr"))
            if b >= 2:
                DVE.wait(("act", "red", b - 2))   # prodP slot reuse
            for m in range(4):
                lo, hi = 4 * m, min(4 * m + 4, NDVE_MUL)
                hb = hidR.ap()[:, b, :].unsqueeze(1).broadcast_to(
                    (128, hi - lo, H))
                DVE.wait(("dma", "Lq", b, m))
                DVE.op(lambda b=b, lo=lo, hi=hi, hb=hb: v.tensor_mul(
                    prodP[b % 2].ap()[:, lo:hi, :],
                    L[b % 2].ap()[:, lo:hi, :], hb),
                    drain=False)
                DVE.mark(f"mA{m + 1}", b)
            for t in range(NACT_RED, T):
                if t == NDVE_MUL:
                    DVE.wait(("gps", "gmA", b))
                if t == NDVE_MUL + 1:
                    DVE.wait(("gps", "gmB", b))
                if t < NDVE_MUL:
                    srcp = prodP[b % 2].ap()[:, t, :]
                else:
                    srcp = prodG[b % 2].ap()[:, t - NDVE_MUL, :]
                DVE.op(lambda b=b, t=t, srcp=srcp: v.reduce_sum(
                    scores[b].ap()[:, t:t + 1], srcp,
                    axis=mybir.AxisListType.X), drain=False)
            DVE.mark("redD", b)
            DVE.wait(("act", "red", b))
            DVE.op(lambda b=b: v.reduce_max(
                mp[b].ap(), scores[b].ap(), axis=mybir.AxisListType.X))
            DVE.mark("rmax", b)
            DVE.wait(("pe", "transp", b))
            DVE.op(lambda b=b: v.reduce_max(
                negM1[b], mpT, axis=mybir.AxisListType.X, negate=True))
            DVE.mark("rmax2", b)
        # ---- tail ----
        DVE.wait(("pe", "z", BPC - 1))
        DVE.op(lambda: v.reciprocal(rZrow, Zps))
        DVE.mark("recip")
        DVE.wait(("pe", "attnT"))
        DVE.op(lambda: v.tensor_copy(
            CT.ap()[:, HCH:NCH, :], stage_t.ap()[:, 0:4 * HCH].bitcast(BF16)))
        DVE.mark("ctA")
        DVE.wait(("pe", "projdone"))
        DVE.wait(("dma", "bias"))
        DVE.op(lambda: v.tensor_add(out_sb[:, 0:512], pjlo[0:BPC, :], bias[:, 0:512]))
        DVE.mark("bias_lo")
        DVE.op(lambda: v.tensor_add(out_sb[:, 512:1024], pjhi[0:BPC, :],
                                    bias[:, 512:1024]), drain=False)
        DVE.mark("bias_hi")

    def prog_act():
        a = ACT.eng if ACT.emit else None
        Copy = mybir.ActivationFunctionType.Copy
        Exp = mybir.ActivationFunctionType.Exp
        for b in range(BPC):
            # reductions for tiles 0..NACT_RED-1 (start as quarters land)
            for t in range(NACT_RED):
                if t % 4 == 0:
                    ACT.wait(("dve", f"mA{t // 4 + 1}", b))
                ACT.op(lambda b=b, t=t: a.activation(
                    out=dmy.ap().broadcast_to((128, H)),
                    in_=prodP[b % 2].ap()[:, t, :], func=Copy,
                    accum_out=scores[b].ap()[:, t:t + 1]),
                    drain=(t == 0))
            ACT.mark("red", b)
            ACT.wait(("pe", "bcast", b))
            ACT.op(lambda b=b: a.activation(
                out=negM[b].ap(), in_=negM_bc, func=Copy))
            ACT.mark("negMcp", b)
            ACT.op(lambda b=b: a.activation(
                out=wexpP[b].ap()[:, :, b], in_=scores[b].ap(), func=Exp,
                bias=negM[b].ap(), scale=1.0, accum_out=zp[b].ap()))
            ACT.mark("exp", b)
        # ---- tail ----
        ACT.wait(("pe", "rZvT"))
        ACT.op(lambda: a.activation(out=rZv, in_=rZvT, func=Copy))
        ACT.mark("rzv")
        ACT.wait(("pe", "e2", BPC - 1))
        ACT.op(lambda: a.activation(
            out=attn8.ap()[0:BPC, 0:512], in_=e2lo[0:BPC, :],
            func=Copy, scale=rZv))
        ACT.mark("cplo")
        ACT.op(lambda: a.activation(
            out=attn8.ap()[0:BPC, 512:1024], in_=e2hi[0:BPC, :],
            func=Copy, scale=rZv), drain=False)
        ACT.mark("cphi")

    progs = [
        (GPS, prog_gps), (DMA, prog_dma), (PE, prog_pe),
        (DVE, prog_dve), (ACT, prog_act),
    ]

    for pr, fn in progs:
        pr.begin(emit=False)
        fn()

    counts.clear()
    sem_names = ["pe", "dve", "act", "gps", "hid", "bias", "hidr",
                 "l0", "l1", "wt", "outd"]
    with nc.Block() as block:
        for sn in sem_names:
            sems[sn] = nc.alloc_semaphore(name=f"{sn}_sem")

        @block.gpsimd
        def _(eng):
            GPS.begin(eng=eng, emit=True)
            prog_gps()

        @block.sync
        def _(eng):
            DMA.begin(eng=eng, emit=True)
            prog_dma()

        @block.tensor
        def _(eng):
            PE.begin(eng=eng, emit=True)
            prog_pe()

        @block.vector
        def _(eng):
            DVE.begin(eng=eng, emit=True)
            prog_dve()

        @block.scalar
        def _(eng):
            ACT.begin(eng=eng, emit=True)
            prog_act()

    return nc


def kernel(lstm_output, hidden, W_combine, b_combine):
    global _cached_nc, last_results
    lstm_output = np.asarray(lstm_output, dtype=np.float32)
    hidden = np.asarray(hidden, dtype=np.float32)
    W_combine = np.asarray(W_combine, dtype=np.float32)
    b_combine = np.asarray(b_combine, dtype=np.float32)

    if _cached_nc is None:
        _cached_nc = _build_program()
    nc = _cached_nc

    wt_host = np.ascontiguousarray(W_combine.T).astype(NPBF16)
    in_maps = []
    for i in range(NCORES):
        sl = slice(i * BPC, (i + 1) * BPC)
        in_maps.append({
            "lstm_output": np.ascontiguousarray(lstm_output[sl]).astype(NPBF16),
            "hidden": np.ascontiguousarray(hidden[sl]).astype(NPBF16),
            "w_t": wt_host,
            "b_combine": b_combine,
        })
    res = run_bass_kernel_spmd(nc, in_maps, core_ids=list(range(NCORES)))
    last_results = res
    return np.concatenate([res.results[i]["out"] for i in range(NCORES)], axis=0)


# revision 26
# speedup vs baseline: 1.3038x; 1.0556x over previous
"""Trainium2 Bass kernel for nn_Attention (dense_transformer, ridge regime).

Computation per batch b:
    scores[s]  = <lstm_output[b,s,:], hidden[b,:]>          # [S]
    w          = softmax(scores)                            # [S]
    attn[h]    = sum_s w[s] * lstm_output[b,s,h]            # [H]
    out[b]     = [hidden[b], attn] @ W_combine.T + b_combine

Sharding: data-parallel over batch B=64 across 8 cores (8 batches/core).
W_combine is passed host-transposed (W.T, [2H, H]) and replicated.

v3 (bf16, all-engine balance): all large operands are cast to bf16 on the
host, halving HBM traffic.  Measured per-[128,1024]-tile costs drove the
split: DVE fused mult 0.55us, any reduce ~1.2us, ACT accum 1.41us, GPS mult
2.56us, PE matmul[*,512] 0.38+0.09us.

Per-core dataflow, per batch (16 s-tiles):
  - DMA: L(b) [128,16,1024] bf16, p-major (contiguous per partition),
    double-buffered, issued in 1MiB quarters; W.T after L(1).
  - products L*hidR: DVE fused-8 mult (tiles 0-7), fused-6 (8-13); GPS
    tensor_mul (14-15).
  - row-sums -> scores: DVE reduce_sum (tiles 0-5), ACT Copy+accum (6-15).
  - softmax: DVE rmax -> PE transpose -> DVE rmax2(neg) -> PE bcast ->
    ACT copy -> ACT exp (bias=-max, Z accumulated per partition).
  - einsum2 on PE: M=8 matmuls, lhsT = per-batch zero-padded [128,8] column
    block of wexp, accumulating all 8 batches into one persistent PSUM pair;
    per-batch Z matmul.
  - projection: hidden half spread over mid-stream PE slack, attn half at
    the tail after the end-of-stream attn transposes.
"""

import numpy as np
import ml_dtypes

import concourse.bass as bass
from concourse import bass_isa, library_config, mybir
from concourse.bass_utils import run_bass_kernel_spmd

F32 = mybir.dt.float32
BF16 = mybir.dt.bfloat16
NPBF16 = ml_dtypes.bfloat16

B, S, H = 64, 2048, 1024
NCORES = 8
BPC = B // NCORES          # batches per core
T = S // 128               # s-tiles per batch
NCH = (2 * H) // 128       # 16 chunks of the combined dim
HCH = H // 128             # 8 chunks of one H

NDVE_MUL = 14              # tiles 0..13 multiplied on DVE (rest on GPS)
NACT_RED = 10              # tiles 0..9 reduced on ACT (rest on DVE)

_cached_nc = None
last_results = None


def _build_program():
    nc = bass.Bass()

    lstm_d = nc.declare_dram_parameter("lstm_output", [BPC, S, H], BF16, isOutput=False)
    hid_d = nc.declare_dram_parameter("hidden", [BPC, H], BF16, isOutput=False)
    wt_d = nc.declare_dram_parameter("w_t", [2 * H, H], BF16, isOutput=False)
    b_d = nc.declare_dram_parameter("b_combine", [H], F32, isOutput=False)
    out_d = nc.declare_dram_parameter("out", [BPC, H], F32, isOutput=True)

    # ---- SBUF ----
    L = [nc.alloc_sbuf_tensor(f"L{i}", [128, T, H], BF16) for i in range(2)]
    WT = nc.alloc_sbuf_tensor("WT", [128, NCH, H], BF16)
    hid_t = nc.alloc_sbuf_tensor("hid", [BPC, H], BF16)
    hid = hid_t.ap()
    bias_t = nc.alloc_sbuf_tensor("bias", [BPC, H], F32)
    bias = bias_t.ap()
    out_t = nc.alloc_sbuf_tensor("out_sb", [BPC, H], F32)
    out_sb = out_t.ap()
    hidR = nc.alloc_sbuf_tensor("hidR", [128, BPC, H], BF16)
    prodP = [nc.alloc_sbuf_tensor(f"prodP{i}", [128, NDVE_MUL, H], BF16)
             for i in range(2)]
    prodG = [nc.alloc_sbuf_tensor(f"prodG{i}", [128, T - NDVE_MUL, H], BF16)
             for i in range(2)]
    dmy = nc.alloc_sbuf_tensor("dmy", [128, 1], BF16)
    CT = nc.alloc_sbuf_tensor("CT", [128, NCH, BPC], BF16)
    wexpP = [nc.alloc_sbuf_tensor(f"wexpP{b}", [128, T, BPC], BF16)
             for b in range(BPC)]
    scores = [nc.alloc_sbuf_tensor(f"scores{b}", [128, T], F32) for b in range(BPC)]
    mp = [nc.alloc_sbuf_tensor(f"mp{b}", [128, 1], F32) for b in range(BPC)]
    zp = [nc.alloc_sbuf_tensor(f"zp{b}", [128, 1], F32) for b in range(BPC)]
    negM = [nc.alloc_sbuf_tensor(f"negM{b}", [128, 1], F32) for b in range(BPC)]
    negM1_t = nc.alloc_sbuf_tensor("negM1s", [1, BPC], F32)
    negM1 = [negM1_t.ap()[0:1, b:b + 1] for b in range(BPC)]
    attn8 = nc.alloc_sbuf_tensor("attn8", [BPC, H], BF16)
    rZrow_t = nc.alloc_sbuf_tensor("rZrow", [1, BPC], F32)
    rZrow = rZrow_t.ap()
    rZv_t = nc.alloc_sbuf_tensor("rZv", [BPC, 1], F32)
    rZv = rZv_t.ap()
    ones_col = nc.alloc_sbuf_tensor("ones_col", [1, 128], F32)
    ones128 = nc.alloc_sbuf_tensor("ones128", [128, 1], F32)
    ident = nc.alloc_sbuf_tensor("ident", [128, 128], F32)
    identB = nc.alloc_sbuf_tensor("identB", [128, 128], BF16)

    # ---- PSUM: 8 banks ----
    e2lo_t = nc.alloc_psum_tensor("e2lo", [BPC, 512], F32)
    e2hi_t = nc.alloc_psum_tensor("e2hi", [BPC, 512], F32)
    pjlo_t = nc.alloc_psum_tensor("pjlo", [BPC, 512], F32)
    pjhi_t = nc.alloc_psum_tensor("pjhi", [BPC, 512], F32)
    stage_t = nc.alloc_psum_tensor("stage", [128, 512], F32)
    stage2_t = nc.alloc_psum_tensor("stage2", [128, 512], F32)
    mpT_t = nc.alloc_psum_tensor("mpT", [1, 128], F32)
    zbank_t = nc.alloc_psum_tensor("zbank", [BPC, 64], F32)
    e2lo, e2hi = e2lo_t.ap(), e2hi_t.ap()
    pjlo, pjhi = pjlo_t.ap(), pjhi_t.ap()
    stage, stage2 = stage_t.ap(), stage2_t.ap()
    mpT = mpT_t.ap()
    negM_bc = stage2_t.ap()[:, 0:1]
    Zps = zbank_t.ap()[0:1, 0:BPC]
    rZvT = zbank_t.ap()[0:BPC, 8:9]
    ctT = [stage_t.ap()[:, 4 * c:4 * (c + 1)].bitcast(BF16) for c in range(HCH)]

    # ---------------- two-pass emission ----------------
    ev = {}
    sems = {}
    counts = {}

    class Prog:
        def __init__(self, name):
            self.name = name
            self.emit = False
            self.eng = None
            self.hwm = {}
            self.auto_drain = name in ("dve", "act", "gps")
            self.first_op = True

        def begin(self, eng=None, emit=False):
            self.emit = emit
            self.eng = eng
            self.hwm = {}
            self.first_op = True

        def wait(self, key):
            if len(key) == 2 and isinstance(key[1], int) and key[0] in (
                    "pe", "dve", "act", "gps", "hid", "bias", "hidr",
                    "l0", "l1", "wt", "outd"):
                sname, val = key
            else:
                if self.emit and key not in ev:
                    raise KeyError(f"wait on unknown event {key}")
                sname, val = ev.get(key, (None, 0))
            if val <= 0 or sname is None:
                return
            if self.hwm.get(sname, -1) >= val:
                return
            self.hwm[sname] = val
            if self.emit:
                self.eng.wait_ge(sems[sname], val)

        def op(self, fn, inc=1, sem=None, drain=None):
            sname = sem or self.name
            counts[sname] = counts.get(sname, 0) + inc
            if self.emit:
                do_drain = self.auto_drain if drain is None else drain
                if do_drain and not self.first_op:
                    self.eng.drain()
                inst = fn()
                inst.then_inc(sems[sname], inc)
            self.first_op = False

        def mark(self, *key, sem=None):
            sname = sem or self.name
            ev[(self.name,) + tuple(key)] = (sname, counts.get(sname, 0))

    DMA, PE, DVE, ACT, GPS = Prog("dma"), Prog("pe"), Prog("dve"), Prog("act"), Prog("gps")

    bias_src = b_d[:]
    bias_bcast = bass.AP(
        tensor=bias_src.tensor,
        offset=bias_src.offset,
        ap=[[0, BPC]] + list(bias_src.ap),
    )

    def prog_gps():
        g = GPS.eng if GPS.emit else None
        GPS.op(lambda: g.memset(ones_col.ap(), 1.0))
        GPS.op(lambda: g.memset(ones128.ap(), 1.0))
        GPS.op(lambda: g.memset(ident.ap(), 0.0))
        GPS.op(lambda: g.affine_select(
            out=ident.ap(), in_=ident.ap(),
            compare_op=mybir.AluOpType.not_equal, fill=1.0, base=0,
            pattern=[[-1, 128]], channel_multiplier=1))
        GPS.op(lambda: g.memset(identB.ap(), 0.0), drain=True)
        GPS.op(lambda: g.affine_select(
            out=identB.ap(), in_=identB.ap(),
            compare_op=mybir.AluOpType.not_equal, fill=1.0, base=0,
            pattern=[[-1, 128]], channel_multiplier=1), drain=True)
        for b in range(BPC):
            GPS.op(lambda b=b: g.memset(wexpP[b].ap(), 0.0), drain=False)
        GPS.mark("setup")
        # per-batch: products for tiles NDVE_MUL..15
        for b in range(BPC):
            GPS.wait(("dma", "Lq", b, 3))
            GPS.wait(("dma", "hidr"))
            if b >= 2:
                GPS.wait(("dve", "redD", b - 2))   # prodG slot reuse
            for j in range(T - NDVE_MUL):
                GPS.op(lambda b=b, j=j: g.tensor_mul(
                    prodG[b % 2].ap()[:, j, :],
                    L[b % 2].ap()[:, NDVE_MUL + j, :],
                    hidR.ap()[:, b, :]), drain=False)
                if j == 0:
                    GPS.mark("gmA", b)
            GPS.mark("gmB", b)

    def prog_dma():
        d = DMA.eng if DMA.emit else None
        DMA.op(lambda: d.dma_start(out=hid, in_=hid_d[:]), inc=16, sem="hid")
        DMA.mark("hid", sem="hid")
        DMA.op(lambda: d.dma_start(out=bias, in_=bias_bcast), inc=16, sem="bias")
        DMA.mark("bias", sem="bias")
        hid_src = hid_d[:]
        hidr_bcast = bass.AP(
            tensor=hid_src.tensor,
            offset=hid_src.offset,
            ap=[[0, 128]] + list(hid_src.ap),
        )
        for b in range(BPC):
            if b >= 2:
                DMA.wait(("pe", "e2", b - 2))
            src = lstm_d[b].rearrange("(p t) h -> p t h", t=T)
            sl = f"l{b % 2}"
            for q in range(4):
                DMA.op(lambda src=src, b=b, q=q: d.dma_start(
                    out=L[b % 2].ap()[:, 4 * q:4 * (q + 1), :],
                    in_=src[:, 4 * q:4 * (q + 1), :]),
                    inc=16, sem=sl)
                DMA.mark("Lq", b, q, sem=sl)
                if b == 0 and q == 0:
                    DMA.op(lambda: d.dma_start(out=hidR.ap(), in_=hidr_bcast),
                           inc=16, sem="hidr")
                    DMA.mark("hidr", sem="hidr")
            DMA.mark("L", b, sem=sl)
            if b == 1:
                wt_src = wt_d[:].rearrange("(c p) n -> p c n", p=128)
                DMA.op(lambda: d.dma_start(out=WT.ap(), in_=wt_src),
                       inc=16, sem="wt")
                DMA.mark("wt", sem="wt")
        DMA.wait(("dve", "bias_hi"))
        DMA.op(lambda: d.dma_start(out=out_d[:], in_=out_sb), inc=16, sem="outd")
        DMA.wait(("outd", counts.get("outd", 0)))

    def prog_pe():
        p = PE.eng if PE.emit else None
        PE.wait(("gps", "setup"))
        PE.wait(("dma", "hid"))
        for c in range(HCH):
            PE.op(lambda c=c: p.transpose(
                ctT[c], hid[0:BPC, c * 128:(c + 1) * 128],
                identB.ap()[0:BPC, 0:BPC]))
        PE.mark("hidT")
        for b in range(BPC):
            PE.wait(("dve", "rmax", b))
            if b >= 1:
                PE.wait(("dve", "rmax2", b - 1))
            PE.op(lambda b=b: p.transpose(mpT, mp[b].ap(), ident.ap()))
            PE.mark("transp", b)
            PE.wait(("dve", "rmax2", b))
            if b >= 1:
                PE.wait(("act", "negMcp", b - 1))
            PE.op(lambda b=b: p.matmul(
                negM_bc, lhsT=ones_col.ap(), rhs=negM1[b],
                start=True, stop=True))
            PE.mark("bcast", b)
            PE.wait(("act", "exp", b))
            PE.op(lambda b=b: p.matmul(
                Zps[0:1, b:b + 1], lhsT=zp[b].ap(), rhs=ones128.ap(),
                start=True, stop=True, skip_group_check=True))
            PE.mark("z", b)
            PE.wait(("dma", "L", b))
            for t in range(T):
                PE.op(lambda b=b, t=t: p.matmul(
                    e2lo[0:BPC, :],
                    lhsT=wexpP[b].ap()[:, t, :],
                    rhs=L[b % 2].ap()[:, t, 0:512],
                    start=(b == 0 and t == 0), stop=(b == BPC - 1 and t == T - 1),
                    skip_group_check=True))
                PE.op(lambda b=b, t=t: p.matmul(
                    e2hi[0:BPC, :],
                    lhsT=wexpP[b].ap()[:, t, :],
                    rhs=L[b % 2].ap()[:, t, 512:1024],
                    start=(b == 0 and t == 0), stop=(b == BPC - 1 and t == T - 1),
                    skip_group_check=True))
            PE.mark("e2", b)
            if 2 <= b <= 5:
                PE.wait(("dma", "wt"))
                PE.wait(("dve", "cth"))
                for c in (2 * (b - 2), 2 * (b - 2) + 1):
                    PE.op(lambda c=c: p.matmul(
                        pjlo[0:BPC, :], lhsT=CT.ap()[:, c, :],
                        rhs=WT.ap()[:, c, 0:512],
                        start=(c == 0), stop=False, skip_group_check=True))
                    PE.op(lambda c=c: p.matmul(
                        pjhi[0:BPC, :], lhsT=CT.ap()[:, c, :],
                        rhs=WT.ap()[:, c, 512:1024],
                        start=(c == 0), stop=False, skip_group_check=True))
                PE.mark("pjh", b)
        # ---- tail ----
        PE.wait(("dve", "recip"))
        PE.op(lambda: p.transpose(rZvT, rZrow, ones128.ap()[0:1, 0:1]))
        PE.mark("rZvT")
        PE.wait(("act", "cphi"))
        PE.wait(("dve", "cth"))
        for c in range(HCH):
            PE.op(lambda c=c: p.transpose(
                ctT[c], attn8.ap()[0:BPC, c * 128:(c + 1) * 128],
                identB.ap()[0:BPC, 0:BPC]))
        PE.mark("attnT")
        PE.wait(("dve", "ctA"))
        for c in range(HCH, NCH):
            PE.op(lambda c=c: p.matmul(
                pjlo[0:BPC, :], lhsT=CT.ap()[:, c, :],
                rhs=WT.ap()[:, c, 0:512],
                start=False, stop=(c == NCH - 1), skip_group_check=True))
            PE.op(lambda c=c: p.matmul(
                pjhi[0:BPC, :], lhsT=CT.ap()[:, c, :],
                rhs=WT.ap()[:, c, 512:1024],
                start=False, stop=(c == NCH - 1), skip_group_check=True))
        PE.mark("projdone")

    def prog_dve():
        v = DVE.eng if DVE.emit else None
        DVE.wait(("pe", "hidT"))
        DVE.op(lambda: v.tensor_copy(
            CT.ap()[:, 0:HCH, :], stage_t.ap()[:, 0:4 * HCH].bitcast(BF16)))
        DVE.mark("cth")
        for b in range(BPC):
            DVE.wait(("dma", "hidr"))
            if b >= 2:
                DVE.wait(("act", "red", b - 2))   # prodP slot reuse
            for m in range(4):
                lo, hi = 4 * m, min(4 * m + 4, NDVE_MUL)
                hb = hidR.ap()[:, b, :].unsqueeze(1).broadcast_to(
                    (128, hi - lo, H))
                DVE.wait(("dma", "Lq", b, m))
                DVE.op(lambda b=b, lo=lo, hi=hi, hb=hb: v.tensor_mul(
                    prodP[b % 2].ap()[:, lo:hi, :],
                    L[b % 2].ap()[:, lo:hi, :], hb),
                    drain=False)
                DVE.mark(f"mA{m + 1}", b)
            for t in range(NACT_RED, T):
                if t == NDVE_MUL:
                    DVE.wait(("gps", "gmA", b))
                if t == NDVE_MUL + 1:
                    DVE.wait(("gps", "gmB", b))
                if t < NDVE_MUL:
                    srcp = prodP[b % 2].ap()[:, t, :]
                else:
                    srcp = prodG[b % 2].ap()[:, t - NDVE_MUL, :]
                DVE.op(lambda b=b, t=t, srcp=srcp: v.reduce_sum(
                    scores[b].ap()[:, t:t + 1], srcp,
                    axis=mybir.AxisListType.X), drain=False)
            DVE.mark("redD", b)
            DVE.wait(("act", "red", b))
            DVE.op(lambda b=b: v.reduce_max(
                mp[b].ap(), scores[b].ap(), axis=mybir.AxisListType.X))
            DVE.mark("rmax", b)
            DVE.wait(("pe", "transp", b))
            DVE.op(lambda b=b: v.reduce_max(
                negM1[b], mpT, axis=mybir.AxisListType.X, negate=True))
            DVE.mark("rmax2", b)
        # ---- tail ----
        DVE.wait(("pe", "z", BPC - 1))
        DVE.op(lambda: v.reciprocal(rZrow, Zps))
        DVE.mark("recip")
        DVE.wait(("pe", "attnT"))
        DVE.op(lambda: v.tensor_copy(
            CT.ap()[:, HCH:NCH, :], stage_t.ap()[:, 0:4 * HCH].bitcast(BF16)))
        DVE.mark("ctA")
        DVE.wait(("pe", "projdone"))
        DVE.wait(("dma", "bias"))
        DVE.op(lambda: v.tensor_add(out_sb[:, 0:512], pjlo[0:BPC, :], bias[:, 0:512]))
        DVE.mark("bias_lo")
        DVE.op(lambda: v.tensor_add(out_sb[:, 512:1024], pjhi[0:BPC, :],
                                    bias[:, 512:1024]), drain=False)
        DVE.mark("bias_hi")

    def prog_act():
        a = ACT.eng if ACT.emit else None
        Copy = mybir.ActivationFunctionType.Copy
        Exp = mybir.ActivationFunctionType.Exp
        for b in range(BPC):
            # reductions for tiles 0..NACT_RED-1 (start as quarters land)
            for t in range(NACT_RED):
                if t % 4 == 0:
                    ACT.wait(("dve", f"mA{t // 4 + 1}", b))
                ACT.op(lambda b=b, t=t: a.activation(
                    out=dmy.ap().broadcast_to((128, H)),
                    in_=prodP[b % 2].ap()[:, t, :], func=Copy,
                    accum_out=scores[b].ap()[:, t:t + 1]),
                    drain=(t == 0))
            ACT.mark("red", b)
            ACT.wait(("pe", "bcast", b))
            ACT.op(lambda b=b: a.activation(
                out=negM[b].ap(), in_=negM_bc, func=Copy))
            ACT.mark("negMcp", b)
            ACT.op(lambda b=b: a.activation(
                out=wexpP[b].ap()[:, :, b], in_=scores[b].ap(), func=Exp,
                bias=negM[b].ap(), scale=1.0, accum_out=zp[b].ap()))
            ACT.mark("exp", b)
        # ---- tail ----
        ACT.wait(("pe", "rZvT"))
        ACT.op(lambda: a.activation(out=rZv, in_=rZvT, func=Copy))
        ACT.mark("rzv")
        ACT.wait(("pe", "e2", BPC - 1))
        ACT.op(lambda: a.activation(
            out=attn8.ap()[0:BPC, 0:512], in_=e2lo[0:BPC, :],
            func=Copy, scale=rZv))
        ACT.mark("cplo")
        ACT.op(lambda: a.activation(
            out=attn8.ap()[0:BPC, 512:1024], in_=e2hi[0:BPC, :],
            func=Copy, scale=rZv), drain=False)
        ACT.mark("cphi")

    progs = [
        (GPS, prog_gps), (DMA, prog_dma), (PE, prog_pe),
        (DVE, prog_dve), (ACT, prog_act),
    ]

    for pr, fn in progs:
        pr.begin(emit=False)
        fn()

    counts.clear()
    sem_names = ["pe", "dve", "act", "gps", "hid", "bias", "hidr",
                 "l0", "l1", "wt", "outd"]
    with nc.Block() as block:
        for sn in sem_names:
            sems[sn] = nc.alloc_semaphore(name=f"{sn}_sem")

        @block.gpsimd
        def _(eng):
            GPS.begin(eng=eng, emit=True)
            prog_gps()

        @block.sync
        def _(eng):
            DMA.begin(eng=eng, emit=True)
            prog_dma()

        @block.tensor
        def _(eng):
            PE.begin(eng=eng, emit=True)
            prog_pe()

        @block.vector
        def _(eng):
            DVE.begin(eng=eng, emit=True)
            prog_dve()

        @block.scalar
        def _(eng):
            ACT.begin(eng=eng, emit=True)
            prog_act()

    return nc


def kernel(lstm_output, hidden, W_combine, b_combine):
    global _cached_nc, last_results
    lstm_output = np.asarray(lstm_output, dtype=np.float32)
    hidden = np.asarray(hidden, dtype=np.float32)
    W_combine = np.asarray(W_combine, dtype=np.float32)
    b_combine = np.asarray(b_combine, dtype=np.float32)

    if _cached_nc is None:
        _cached_nc = _build_program()
    nc = _cached_nc

    wt_host = np.ascontiguousarray(W_combine.T).astype(NPBF16)
    in_maps = []
    for i in range(NCORES):
        sl = slice(i * BPC, (i + 1) * BPC)
        in_maps.append({
            "lstm_output": np.ascontiguousarray(lstm_output[sl]).astype(NPBF16),
            "hidden": np.ascontiguousarray(hidden[sl]).astype(NPBF16),
            "w_t": wt_host,
            "b_combine": b_combine,
        })
    res = run_bass_kernel_spmd(nc, in_maps, core_ids=list(range(NCORES)))
    last_results = res
    return np.concatenate([res.results[i]["out"] for i in range(NCORES)], axis=0)
